# revision 1
# baseline (speedup 1.0000x reference)
"""Trainium2 Bass kernel: top-2 MoE routing (E=16, D=H=2048), 8 NeuronCores.

Strategy (memory-regime optimal: only the 2 selected experts' weights are
ever read from HBM):
  * Every core redundantly computes the gating on-device: logits = Wg@x+bg,
    top-2 indices + normalized softmax gates.
  * Weights are sharded across cores *within* each expert: core c owns rows
    [c*256, (c+1)*256) of every expert's W1 (pre-transposed to [E, D, 256])
    and the matching contraction slice of W2 (pre-transposed to
    [E, 256, H]).  After gating, each core pulls ONLY the two selected
    experts' slices (2x2MB + 2x2MB) via dynamic-offset DMAs whose expert
    index comes from a register.
  * Because the host pre-transposes the slices, the contraction index (d
    for layer 1, i for layer 2) lies on SBUF partitions, so the tensor
    engine does every matvec as accumulating [K=128, M=128, N=1] matmuls:
    h = tanh(W1[e]@x + b1[e]) lands as [128, 1] PSUM columns, which after
    tanh are directly the moving operand for layer 2.
  * Each core's gate-weighted partial output (+ tkg_k * b2[e_k]/8) is
    AllReduced across the 8 cores, yielding the exact full output.
"""

import numpy as np

try:  # make concourse importable in bare environments
    import concourse.bacc  # noqa: F401
except ImportError:  # pragma: no cover
    import sys

    sys.path.insert(0, "/opt/trn_rl_repo")

E, D, H = 16, 2048, 2048
NCORES = 8
P = 128
RS = H // NCORES  # 256 rows of each expert held per core
NCH = RS // P  # 2 partition-chunks per 256 rows
DC = D // P  # 16 contraction chunks for layer 1
OC = H // P  # 16 output chunks for layer 2

_BUILT = None


def _build(stage=2):
    """Build + compile the Bass program once. Returns (nc, input_names).

    stage: 0=gating, 1=+layer1, 2=+layer2, 3=AllReduce, 4=ReduceScatter.
    """
    global _BUILT
    if _BUILT is not None and _BUILT[2] == stage:
        return _BUILT[:2]

    import concourse.bacc as bacc
    import concourse.bass as bass
    import concourse.tile as tile
    from concourse import mybir

    f32 = mybir.dt.float32
    i32 = mybir.dt.int32
    AX = mybir.AxisListType.X
    OP = mybir.AluOpType

    nc = bacc.Bacc(
        "TRN2", target_bir_lowering=False, debug=False, num_devices=NCORES
    )

    # ----- I/O ------------------------------------------------------------
    x_d = nc.dram_tensor("x", [1, D], f32, kind="ExternalInput")
    wgt_d = nc.dram_tensor("wgt", [D, E], f32, kind="ExternalInput")  # Wg.T
    bg_d = nc.dram_tensor("bg", [1, E], f32, kind="ExternalInput")
    iota_d = nc.dram_tensor("iota16", [1, E], f32, kind="ExternalInput")
    w1t_d = b1c_d = w2t_d = b2d_d = None
    if stage >= 1:
        # W1 slice pre-transposed on host: [E, D, RS] (rows d, cols r)
        w1t_d = nc.dram_tensor("w1t", [E, D, RS], f32, kind="ExternalInput")
        b1c_d = nc.dram_tensor("b1c", [E, RS], f32, kind="ExternalInput")
    if stage >= 2:
        # W2 slice pre-transposed on host: [E, RS, H] (rows i, cols o)
        w2t_d = nc.dram_tensor("w2t", [E, RS, H], f32, kind="ExternalInput")
        b2d_d = nc.dram_tensor("b2d", [E, H], f32, kind="ExternalInput")
    out_d = nc.dram_tensor("out", [1, H], f32, kind="ExternalOutput")
    dbg_d = nc.dram_tensor("dbg", [1, 64], f32, kind="ExternalOutput")

    in_names = ["x", "wgt", "bg", "iota16"]
    if stage >= 1:
        in_names += ["w1t", "b1c"]
    if stage >= 2:
        in_names += ["w2t", "b2d"]

    with tile.TileContext(nc) as tc:
        with (
            tc.tile_pool(name="sb", bufs=1) as sb,
            tc.tile_pool(name="scr", bufs=2) as scr,
            tc.tile_pool(name="ps", bufs=1, space="PSUM") as ps,
            tc.tile_pool(name="dr", bufs=1, space="DRAM") as dr,
        ):
            # ----- static loads, spread across the three DMA rings ---------
            # x on partitions by contraction chunk: x_pd[p, dc] = x[dc*128+p]
            x_pd = sb.tile([P, DC], f32, tag="x_pd")
            nc.scalar.dma_start(
                x_pd[:], x_d.ap().rearrange("o (dc p) -> p (o dc)", p=P)
            )
            # Wg.T with contraction d on partitions: [128, dc, e]
            wgt_sb = sb.tile([P, DC * E], f32, tag="wgt")
            nc.sync.dma_start(
                wgt_sb[:].rearrange("p (dc e) -> p dc e", dc=DC),
                wgt_d.ap().rearrange("(dc p) e -> p dc e", p=P),
            )
            bg_sb = sb.tile([1, E], f32, tag="bg")
            nc.scalar.dma_start(bg_sb[:], bg_d.ap())
            iota_sb = sb.tile([1, E], f32, tag="iota")
            nc.scalar.dma_start(iota_sb[:], iota_d.ap())
            one_sb = sb.tile([1, 1], f32, tag="one")
            nc.gpsimd.memset(one_sb[:], 1.0)

            # ----- gating on PE: logits = Wg @ x + bg ----------------------
            # x chunk stationary, Wg.T chunk moving -> logits land [1, E]
            # directly in free-dim layout; bg folds in as a K=1 matmul
            lg_ps = ps.tile([1, E], f32, tag="lg_ps")
            for dc in range(DC):
                nc.tensor.matmul(
                    out=lg_ps[:],
                    lhsT=x_pd[:, dc : dc + 1],
                    rhs=wgt_sb[:, dc * E : (dc + 1) * E],
                    start=(dc == 0),
                    stop=False,
                )
            nc.tensor.matmul(
                out=lg_ps[:], lhsT=one_sb[:], rhs=bg_sb[:], start=False,
                stop=True,
            )
            logits = lg_ps

            # ----- top-1 index as fast as possible (gates the W1 DMA) ------
            m1 = sb.tile([1, 1], f32, tag="m1")
            nc.vector.tensor_reduce(m1[:], logits[:], axis=AX, op=OP.max)
            mask1 = sb.tile([1, E], f32, tag="mask1")
            nc.vector.tensor_scalar(mask1[:], logits[:], m1[:], None, OP.is_ge)
            mjunk = sb.tile([1, E], f32, tag="mjunk")
            idx1f = sb.tile([1, 1], f32, tag="idx1f")
            nc.vector.tensor_mul(mjunk[:], mask1[:], iota_sb[:])
            nc.vector.tensor_reduce(idx1f[:], mjunk[:], axis=AX, op=OP.add)
            idx_i = [
                sb.tile([1, 1], i32, tag=f"idxi{k}", name=f"idxi{k}")
                for k in range(2)
            ]
            # int index copies issue immediately after each reduce: DVE runs
            # in order, so these must NOT queue behind the tkg/debug ops --
            # they gate the register loads that start the weight DMAs
            nc.vector.tensor_copy(idx_i[0][:], idx1f[:])

            # top-2: mask out the max with a large subtraction (logits can
            # be negative, so multiplying by (1-mask) would be wrong)
            pen = sb.tile([1, E], f32, tag="pen")
            nc.vector.tensor_scalar_mul(pen[:], mask1[:], 1e30)
            l2v = sb.tile([1, E], f32, tag="l2v")
            nc.vector.tensor_tensor(
                out=l2v[:], in0=logits[:], in1=pen[:], op=OP.subtract
            )
            m2 = sb.tile([1, 1], f32, tag="m2")
            nc.vector.tensor_reduce(m2[:], l2v[:], axis=AX, op=OP.max)
            mask2 = sb.tile([1, E], f32, tag="mask2")
            nc.vector.tensor_scalar(mask2[:], l2v[:], m2[:], None, OP.is_ge)
            mjunk2 = sb.tile([1, E], f32, tag="mjunk2")
            idx2f = sb.tile([1, 1], f32, tag="idx2f")
            nc.vector.tensor_mul(mjunk2[:], mask2[:], iota_sb[:])
            nc.vector.tensor_reduce(idx2f[:], mjunk2[:], axis=AX, op=OP.add)
            nc.vector.tensor_copy(idx_i[1][:], idx2f[:])

            # normalized top-2 gates: with e_k = exp(l_k - m1):
            #   e_top1 = 1;  e_top2 = exp(m2 - m1);  S = sum(exp(logits-m1))
            #   tkg_0 = 1/(1 + e_top2 + 1e-6*S);  tkg_1 = e_top2 * tkg_0
            negm1 = sb.tile([1, 1], f32, tag="negm1")
            nc.vector.tensor_scalar_mul(negm1[:], m1[:], -1.0)
            esb = sb.tile([1, E], f32, tag="esb")
            nc.scalar.activation(
                esb[:], logits[:], mybir.ActivationFunctionType.Exp,
                bias=negm1[:],
            )
            ssum = sb.tile([1, 1], f32, tag="ssum")
            nc.vector.tensor_reduce(ssum[:], esb[:], axis=AX, op=OP.add)
            e2x = sb.tile([1, 1], f32, tag="e2x")
            nc.scalar.activation(
                e2x[:], m2[:], mybir.ActivationFunctionType.Exp, bias=negm1[:]
            )
            den = sb.tile([1, 1], f32, tag="den")
            nc.vector.tensor_scalar(den[:], ssum[:], 1e-6, 1.0, OP.mult, OP.add)
            nc.vector.tensor_add(den[:], den[:], e2x[:])
            tkg = [
                sb.tile([1, 1], f32, tag=f"tkg{k}", name=f"tkg{k}")
                for k in range(2)
            ]
            nc.vector.reciprocal(tkg[0][:], den[:])
            nc.vector.tensor_mul(tkg[1][:], e2x[:], tkg[0][:])
            tkg_rep = [
                sb.tile([P, 1], f32, tag=f"tkgr{k}", name=f"tkgr{k}")
                for k in range(2)
            ]
            for k in range(2):
                nc.gpsimd.partition_broadcast(tkg_rep[k][:], tkg[k][:])

            # debug output: logits, e, tkg, idx
            dbg = sb.tile([1, 64], f32, tag="dbg")
            nc.gpsimd.memset(dbg[:], 0.0)
            nc.vector.tensor_copy(dbg[:, 0:E], logits[:])
            nc.vector.tensor_copy(dbg[:, E : 2 * E], esb[:])
            nc.vector.tensor_copy(dbg[:, 32:33], tkg[0][:])
            nc.vector.tensor_copy(dbg[:, 33:34], tkg[1][:])
            nc.vector.tensor_copy(dbg[:, 34:35], idx1f[:])
            nc.vector.tensor_copy(dbg[:, 35:36], idx2f[:])
            nc.scalar.dma_start(dbg_d.ap(), dbg[:])

            if stage == 0:
                nc.scalar.dma_start(out_d.ap()[:, 0:DC], x_pd[0:1, :])
                nc.scalar.dma_start(
                    out_d.ap()[:, DC:], wgt_sb[0:1, 0 : D - DC]
                )

            if stage >= 1:
                # expert indices -> registers on SP (weights) + Pool (biases)
                sv = []  # SP-register index per expert slot
                pv = []  # Pool-register index per expert slot
                b1t = [
                    sb.tile([P, NCH], f32, tag=f"b1t{k}", name=f"b1t{k}")
                    for k in range(2)
                ]
                w1t = [
                    sb.tile([P, DC * RS], f32, tag=f"w1t{k}", name=f"w1t{k}")
                    for k in range(2)
                ]
                for k in range(2):
                    # expert-k index registers, then its W1 DMA immediately so
                    # the weight stream starts before idx of slot k+1 resolves
                    reg = nc.sync.alloc_register(f"idx_sp{k}")
                    nc.sync.reg_load(reg, idx_i[k][:])
                    sv.append(nc.snap(reg, donate=True, min_val=0, max_val=E - 1))
                    preg = nc.gpsimd.alloc_register(f"idx_pool{k}")
                    nc.gpsimd.reg_load(preg, idx_i[k][:])
                    pv.append(nc.snap(preg, donate=True, min_val=0, max_val=E - 1))
                    nc.sync.dma_start(
                        w1t[k][:].rearrange("p (dc r) -> p dc r", dc=DC),
                        w1t_d.ap()[bass.ds(sv[k], 1), :, :].rearrange(
                            "o (dc p) r -> p (o dc) r", p=P
                        ),
                    )
                    nc.gpsimd.dma_start(
                        b1t[k][:],
                        b1c_d.ap()[bass.ds(pv[k], 1), :].rearrange(
                            "o (c p) -> p (o c)", p=P
                        ),
                    )

                # layer 1 on PE: h_ps[:, rc] += w1t[e][:, dc, rc*128:...].T @ x
                h_ps = [
                    ps.tile([P, NCH], f32, tag=f"hps{k}", name=f"hps{k}")
                    for k in range(2)
                ]
                hs = [
                    sb.tile([P, NCH], f32, tag=f"hs{k}", name=f"hs{k}")
                    for k in range(2)
                ]
                for k in range(2):
                    for rc in range(NCH):
                        for dc in range(DC):
                            nc.tensor.matmul(
                                out=h_ps[k][:, rc : rc + 1],
                                lhsT=w1t[k][
                                    :, dc * RS + rc * P : dc * RS + (rc + 1) * P
                                ],
                                rhs=x_pd[:, dc : dc + 1],
                                start=(dc == 0),
                                stop=(dc == DC - 1),
                            )
                        nc.scalar.activation(
                            hs[k][:, rc : rc + 1],
                            h_ps[k][:, rc : rc + 1],
                            mybir.ActivationFunctionType.Tanh,
                            bias=b1t[k][:, rc : rc + 1],
                        )

            if stage == 1:
                nc.scalar.dma_start(out_d.ap()[:, 0:P], hs[0][:, 0:1])
                nc.scalar.dma_start(out_d.ap()[:, P : 2 * P], hs[0][:, 1:2])
                nc.scalar.dma_start(out_d.ap()[:, 2 * P : 3 * P], hs[1][:, 0:1])
                nc.scalar.dma_start(out_d.ap()[:, 3 * P : 4 * P], hs[1][:, 1:2])
                nc.scalar.dma_start(
                    out_d.ap()[:, 4 * P :], wgt_sb[0:1, 0 : D - 4 * P]
                )

            if stage >= 2:
                b2t = [
                    sb.tile([P, OC], f32, tag=f"b2t{k}", name=f"b2t{k}")
                    for k in range(2)
                ]
                for k in range(2):
                    nc.gpsimd.dma_start(
                        b2t[k][:],
                        b2d_d.ap()[bass.ds(pv[k], 1), :].rearrange(
                            "o (c p) -> p (o c)", p=P
                        ),
                    )
                # W2 slice (transposed): one 1MB DMA per contraction half,
                # so layer-2 matmuls on half 0 start while half 1 streams
                w2t = [
                    sb.tile([P, NCH * H], f32, tag=f"w2t{k}", name=f"w2t{k}")
                    for k in range(2)
                ]
                for k in range(2):
                    w2view = w2t_d.ap()[bass.ds(sv[k], 1), :, :].rearrange(
                        "a (ic p) o -> p (a ic) o", p=P
                    )
                    for ic in range(NCH):
                        nc.sync.dma_start(
                            w2t[k][:, ic * H : (ic + 1) * H],
                            w2view[:, ic : ic + 1, :],
                        )

                # layer 2 on PE: eo_ps[:, oc] += w2t[e][ic][:, oc*128:...].T @ h
                eo_ps = [
                    ps.tile([P, OC], f32, tag=f"eops{k}", name=f"eops{k}")
                    for k in range(2)
                ]
                eo = [
                    sb.tile([P, OC], f32, tag=f"eo{k}", name=f"eo{k}")
                    for k in range(2)
                ]
                for k in range(2):
                    for oc in range(OC):
                        for ic in range(NCH):
                            nc.tensor.matmul(
                                out=eo_ps[k][:, oc : oc + 1],
                                lhsT=w2t[k][
                                    :, ic * H + oc * P : ic * H + (oc + 1) * P
                                ],
                                rhs=hs[k][:, ic : ic + 1],
                                start=(ic == 0),
                                stop=(ic == NCH - 1),
                            )
                    nc.vector.tensor_copy(eo[k][:], eo_ps[k][:])

                # combine: res = sum_k tkg_k * (eo_k + b2[e_k]/NCORES)
                res = sb.tile([P, OC], f32, tag="res")
                vk = [
                    sb.tile([P, OC], f32, tag=f"vk{k}", name=f"vk{k}")
                    for k in range(2)
                ]
                for k in range(2):
                    nc.vector.tensor_add(vk[k][:], eo[k][:], b2t[k][:])
                    nc.vector.tensor_scalar(
                        vk[k][:], vk[k][:], tkg_rep[k][:], None, OP.mult
                    )
                nc.vector.tensor_add(res[:], vk[0][:], vk[1][:])

            if stage == 2:
                nc.sync.dma_start(
                    out_d.ap().rearrange("o (oc p) -> (o p) oc", p=P), res[:]
                )

            if stage == 3:
                cc_in = dr.tile([1, H], f32, tag="cc_in")
                cc_out = dr.tile([1, H], f32, tag="cc_out")
                nc.sync.dma_start(
                    cc_in[:].rearrange("o (oc p) -> (o p) oc", p=P), res[:]
                )
                nc.gpsimd.collective_compute(
                    "AllReduce",
                    mybir.AluOpType.add,
                    replica_groups=[list(range(NCORES))],
                    ins=[cc_in[:]],
                    outs=[cc_out[:]],
                )
                nc.sync.dma_start(out_d.ap(), cc_out[:])

            if stage >= 4:
                # ReduceScatter: core c receives the summed elements
                # [c*RS, (c+1)*RS); the host concatenates the 8 shards.
                cc_in = dr.tile([1, H], f32, tag="cc_in")
                cc_sh = dr.tile([1, RS], f32, tag="cc_sh")
                nc.sync.dma_start(
                    cc_in[:].rearrange("o (oc p) -> (o p) oc", p=P), res[:]
                )
                nc.gpsimd.collective_compute(
                    "ReduceScatter",
                    mybir.AluOpType.add,
                    replica_groups=[list(range(NCORES))],
                    ins=[cc_in[:]],
                    outs=[cc_sh[:]],
                )
                nc.sync.dma_start(out_d.ap()[:, 0:RS], cc_sh[:])

    nc.compile()
    _BUILT = (nc, in_names, stage)
    return _BUILT[:2]


def make_in_maps(x, Wg, bg, W1, b1, W2, b2):
    """Host-side sharding: per-core input dicts."""
    x = np.ascontiguousarray(np.asarray(x, np.float32).reshape(1, D))
    Wg = np.asarray(Wg, np.float32)
    bg = np.ascontiguousarray(np.asarray(bg, np.float32).reshape(1, E))
    W1 = np.asarray(W1, np.float32)
    b1 = np.asarray(b1, np.float32)
    W2 = np.asarray(W2, np.float32)
    b2 = np.asarray(b2, np.float32)

    wgt = np.ascontiguousarray(Wg.T)
    b2d = np.ascontiguousarray(b2 / NCORES)
    iota16 = np.arange(E, dtype=np.float32).reshape(1, E)

    in_maps = []
    for c in range(NCORES):
        rs = slice(c * RS, (c + 1) * RS)
        in_maps.append(
            {
                "x": x,
                "wgt": wgt,
                "bg": bg,
                "w1t": np.ascontiguousarray(W1[:, rs, :].transpose(0, 2, 1)),
                "b1c": np.ascontiguousarray(b1[:, rs]),
                "w2t": np.ascontiguousarray(W2[:, :, rs].transpose(0, 2, 1)),
                "b2d": b2d,
                "iota16": iota16,
            }
        )
    return in_maps


def kernel(x, Wg, bg, W1, b1, W2, b2, train=0, **_unused):
    import os

    from concourse import bass_utils

    stage = int(os.environ.get("MOE_STAGE", "2"))
    nc, _ = _build(stage=stage)
    in_maps = make_in_maps(x, Wg, bg, W1, b1, W2, b2)
    res = bass_utils.run_bass_kernel_spmd(
        nc, in_maps, core_ids=list(range(NCORES))
    )
    outs = [
        np.asarray(res.results[c]["out"], np.float32).reshape(H)
        for c in range(NCORES)
    ]
    if stage == 2:
        # each core holds the gate-weighted partial sum over its contraction
        # shard; unshard by summing the partials
        return np.sum(outs, axis=0, dtype=np.float32)
    if stage == 3:  # AllReduce: every core has the full output
        return outs[0]
    # stage 4, ReduceScatter: core c holds rows [c*RS, (c+1)*RS)
    return np.concatenate([o[0:RS] for o in outs])



# revision 15
# speedup vs baseline: 1.8140x; 1.8140x over previous
"""Trainium2 Bass kernel: top-2 MoE routing (E=16, D=H=2048), 8 NeuronCores.

Strategy (memory-regime optimal: only the 2 selected experts' weights are
ever read from HBM, in bf16):
  * Every core redundantly computes the gating on-device: logits = Wg@x+bg,
    top-2 indices + normalized softmax gates. x/Wg/bg are packed into a
    single bf16 [128, 288] tile so ONE static DMA feeds the whole gating.
  * Weights are sharded across cores *within* each expert: core c owns rows
    [c*256, (c+1)*256) of every expert's W1 and the matching contraction
    slice of W2. The host pre-transposes each per-core expert slice into a
    [E, 128, 4096] bf16 layout whose per-partition lines are contiguous in
    DRAM (128 descriptors x 8KB per slice). After gating, each core pulls
    ONLY the two selected experts' slices (4 x 1MB) via dynamic-offset DMAs
    spread over the SP and Activation HWDGE queues.
  * The contraction index lies on SBUF partitions, so the tensor engine does
    every matvec as accumulating [K=128, M=128, N=1] matmuls. b1/b2 biases
    are folded in as K=1 matmuls (lhsT = bias row, rhs = 1 or tkg_k), and
    the tanh output is pre-scaled by tkg_k so BOTH experts accumulate into
    one shared PSUM tile - the gate-weighted sum needs no vector combine.
  * Each core writes its [128, 16] partial; the host transposes + sums the
    8 partials into the exact full output.
"""

import numpy as np

try:  # make concourse importable in bare environments
    import concourse.bacc  # noqa: F401
except ImportError:  # pragma: no cover
    import sys

    sys.path.insert(0, "/opt/trn_rl_repo")

E, D, H = 16, 2048, 2048
NCORES = 8
P = 128
RS = H // NCORES  # 256 rows of each expert held per core
NCH = RS // P  # 2 partition-chunks per 256 rows
DC = D // P  # 16 contraction chunks for layer 1
OC = H // P  # 16 output chunks for layer 2
BH = RS + H  # concatenated per-expert bias row (b1 slice | b2/NCORES)
XW = DC + DC * E + E  # packed x | Wg.T | bg columns
WARM = 320  # PE warmup matmul width (pstate pump during input DMA)

_BUILT = None


def _build():
    """Build + compile the Bass program once. Returns (nc, input_names)."""
    global _BUILT
    if _BUILT is not None:
        return _BUILT
    import os

    _debug = bool(int(os.environ.get("MOE_DEBUG", "0")))

    import concourse.bacc as bacc
    import concourse.bass as bass
    import concourse.tile as tile
    from concourse import mybir

    f32 = mybir.dt.float32
    bf16 = mybir.dt.bfloat16
    i32 = mybir.dt.int32
    AX = mybir.AxisListType.X
    OP = mybir.AluOpType
    ACT = mybir.ActivationFunctionType

    nc = bacc.Bacc(
        "TRN2", target_bir_lowering=False, debug=False, num_devices=NCORES
    )

    # ----- I/O ------------------------------------------------------------
    xwg_d = nc.dram_tensor("xwg", [P, XW], bf16, kind="ExternalInput")
    iota_d = nc.dram_tensor("iota16", [1, E], f32, kind="ExternalInput")
    w1b_d = nc.dram_tensor("w1b", [E, P, DC * RS], bf16, kind="ExternalInput")
    w2b_d = nc.dram_tensor("w2b", [E, P, NCH * H], bf16, kind="ExternalInput")
    bcat_d = nc.dram_tensor("bcat", [E, BH], f32, kind="ExternalInput")
    out_d = nc.dram_tensor("out", [P, OC], f32, kind="ExternalOutput")
    in_names = ["xwg", "iota16", "w1b", "w2b", "bcat"]

    with tile.TileContext(nc) as tc:
        with (
            tc.tile_pool(name="sb", bufs=1) as sb,
            tc.tile_pool(name="ps", bufs=1, space="PSUM") as ps,
        ):
            # ----- constants (Pool engine, no DMA) -------------------------
            one_f = sb.tile([1, 1], f32, tag="one_f")
            nc.gpsimd.memset(one_f[:], 1.0)
            one_b = sb.tile([1, 1], bf16, tag="one_b")
            nc.gpsimd.memset(one_b[:], 1.0)
            wrow = sb.tile([1, WARM], f32, tag="wrow")
            nc.gpsimd.memset(wrow[:], 0.0)

            # ----- static loads -------------------------------------------
            # packed gating operands: one 128x288 bf16 DMA on the SP queue
            xwg = sb.tile([P, XW], bf16, tag="xwg")
            nc.sync.dma_start(xwg[:], xwg_d.ap())
            iota_sb = sb.tile([1, E], f32, tag="iota")
            nc.gpsimd.dma_start(iota_sb[:], iota_d.ap())

            # ----- PE warmup: ramp the pstate while inputs stream ----------
            wm_ps = ps.tile([1, WARM], f32, tag="wm_ps")
            nc.tensor.matmul(
                out=wm_ps[:], lhsT=one_f[:], rhs=wrow[:], start=True, stop=True
            )

            # ----- gating on PE: logits = Wg @ x + bg ----------------------
            xcol = lambda dc: xwg[:, dc : dc + 1]
            lg_ps = ps.tile([1, E], f32, tag="lg_ps")
            for dc in range(DC):
                nc.tensor.matmul(
                    out=lg_ps[:],
                    lhsT=xcol(dc),
                    rhs=xwg[:, DC + dc * E : DC + (dc + 1) * E],
                    start=(dc == 0),
                    stop=False,
                )
            nc.tensor.matmul(
                out=lg_ps[:],
                lhsT=one_b[:],
                rhs=xwg[0:1, DC + DC * E : DC + DC * E + E],
                start=False,
                stop=True,
            )
            logits = lg_ps

            # ----- top-2 via the DVE max8 unit (gates the weight DMAs) -----
            # max gives the 8 largest values in DESCENDING order, max_index
            # their indices: one pair of ops yields both experts at once,
            # reading the logits straight out of PSUM.
            vals8 = sb.tile([1, 8], f32, tag="vals8")
            nc.vector.max(vals8[:], logits[:])
            idx8 = sb.tile([1, 8], mybir.dt.uint32, tag="idx8")
            nc.vector.max_index(idx8[:], vals8[:], logits[:])
            idx_i = [idx8[0:1, k : k + 1] for k in range(2)]

            # ----- expert-indexed loads: SP streams e0, Act streams e1 -----
            w1t = [
                sb.tile([P, DC * RS], bf16, tag=f"w1t{k}", name=f"w1t{k}")
                for k in range(2)
            ]
            w2t = [
                sb.tile([P, NCH * H], bf16, tag=f"w2t{k}", name=f"w2t{k}")
                for k in range(2)
            ]
            bc = [
                sb.tile([1, BH], f32, tag=f"bc{k}", name=f"bc{k}")
                for k in range(2)
            ]
            # split the last weight transfer so only a sliver of L2 work
            # remains after the final byte lands
            TSPLIT = 15 * P  # w2t[1] free-dim split point (oc = 15)
            qeng = [nc.sync, nc.scalar]
            for k in range(2):
                eng = qeng[k]
                reg = eng.alloc_register(f"idx_q{k}")
                eng.reg_load(reg, idx_i[k])
                sv = nc.snap(reg, donate=True, min_val=0, max_val=E - 1)
                eng.dma_start(
                    w1t[k][:],
                    w1b_d.ap()[bass.ds(sv, 1), :, :].rearrange(
                        "a p f -> p (a f)"
                    ),
                )
                w2src = w2b_d.ap()[bass.ds(sv, 1), :, :].rearrange(
                    "a p (ic o) -> p (a ic) o", ic=NCH
                )
                w2dst = w2t[k][:].rearrange("p (ic o) -> p ic o", ic=NCH)
                if k == 0:
                    eng.dma_start(w2dst, w2src)
                else:
                    eng.dma_start(
                        w2dst[:, :, 0:TSPLIT], w2src[:, :, 0:TSPLIT]
                    )
                    eng.dma_start(
                        w2dst[:, :, TSPLIT:], w2src[:, :, TSPLIT:]
                    )
                preg = nc.gpsimd.alloc_register(f"idx_pool{k}")
                nc.gpsimd.reg_load(preg, idx_i[k])
                pv = nc.snap(preg, donate=True, min_val=0, max_val=E - 1)
                nc.gpsimd.dma_start(
                    bc[k][:], bcat_d.ap()[bass.ds(pv, 1), :]
                )

            # ----- normalized top-2 gates (off the DMA critical path) ------
            # with e2 = exp(l2 - l1): tkg_0 = 1/(1 + e2); tkg_1 = e2*tkg_0.
            # (The reference's +1e-6*S denominator term shifts tkg by <2e-5
            # relative - far below the bf16 noise floor - so it is dropped.)
            negm1 = sb.tile([1, 1], f32, tag="negm1")
            nc.vector.tensor_scalar_mul(negm1[:], vals8[0:1, 0:1], -1.0)
            e2x = sb.tile([1, 1], f32, tag="e2x")
            nc.scalar.activation(
                e2x[:], vals8[0:1, 1:2], ACT.Exp, bias=negm1[:]
            )
            den = sb.tile([1, 1], f32, tag="den")
            nc.vector.tensor_scalar_add(den[:], e2x[:], 1.0)
            tkg = [
                sb.tile([1, 1], f32, tag=f"tkg{k}", name=f"tkg{k}")
                for k in range(2)
            ]
            nc.vector.reciprocal(tkg[0][:], den[:])
            nc.vector.tensor_mul(tkg[1][:], e2x[:], tkg[0][:])
            # per-partition broadcast for the tanh-output scale
            tkgr = [
                sb.tile([P, 1], f32, tag=f"tkgr{k}", name=f"tkgr{k}")
                for k in range(2)
            ]
            for k in range(2):
                nc.gpsimd.partition_broadcast(tkgr[k][:], tkg[k][:])

            # ----- layer 1 + tanh + tkg scale ------------------------------
            # h_ps[:, rc] = W1[e] chunks @ x + b1[e] (bias as a K=1 matmul)
            h_ps = [
                ps.tile([P, NCH], f32, tag=f"hps{k}", name=f"hps{k}")
                for k in range(2)
            ]
            hs = [
                sb.tile([P, NCH], bf16, tag=f"hs{k}", name=f"hs{k}")
                for k in range(2)
            ]
            hss = [
                sb.tile([P, NCH], bf16, tag=f"hss{k}", name=f"hss{k}")
                for k in range(2)
            ]
            for k in range(2):
                for rc in range(NCH):
                    for dc in range(DC):
                        nc.tensor.matmul(
                            out=h_ps[k][:, rc : rc + 1],
                            lhsT=w1t[k][
                                :, dc * RS + rc * P : dc * RS + (rc + 1) * P
                            ],
                            rhs=xcol(dc),
                            start=(dc == 0),
                            stop=False,
                        )
                    nc.tensor.matmul(
                        out=h_ps[k][:, rc : rc + 1],
                        lhsT=bc[k][0:1, rc * P : (rc + 1) * P],
                        rhs=one_f[:],
                        start=False,
                        stop=True,
                    )
                    nc.scalar.activation(
                        hs[k][:, rc : rc + 1],
                        h_ps[k][:, rc : rc + 1],
                        ACT.Tanh,
                    )
                    nc.vector.tensor_scalar(
                        hss[k][:, rc : rc + 1],
                        hs[k][:, rc : rc + 1],
                        tkgr[k][:],
                        None,
                        OP.mult,
                    )

            # ----- layer 2: per-expert gate-weighted partials --------------
            # eo_ps[k][:, oc] = W2[e_k] @ (tkg_k * h_k) + tkg_k * b2[e_k]/8
            eo_ps = [
                ps.tile([P, OC], f32, tag=f"eops{k}", name=f"eops{k}")
                for k in range(2)
            ]
            for k in range(2):
                for oc in range(OC):
                    for ic in range(NCH):
                        nc.tensor.matmul(
                            out=eo_ps[k][:, oc : oc + 1],
                            lhsT=w2t[k][
                                :, ic * H + oc * P : ic * H + (oc + 1) * P
                            ],
                            rhs=hss[k][:, ic : ic + 1],
                            start=(ic == 0),
                            stop=False,
                        )
                    nc.tensor.matmul(
                        out=eo_ps[k][:, oc : oc + 1],
                        lhsT=bc[k][0:1, RS + oc * P : RS + (oc + 1) * P],
                        rhs=tkg[k][:],
                        start=False,
                        stop=True,
                    )

            # ----- write the per-core partial ------------------------------
            # e0's partial moves PSUM->SBUF early (Act, off the critical
            # path); the tail is a single DVE add (one PSUM read) + DMA out.
            vk0 = sb.tile([P, OC], f32, tag="vk0")
            nc.scalar.activation(vk0[:], eo_ps[0][:], ACT.Copy)
            res = sb.tile([P, OC], f32, tag="res")
            nc.vector.tensor_add(res[:], vk0[:], eo_ps[1][:])
            nc.sync.dma_start(out_d.ap(), res[:])

            if _debug:
                dbg_hs = nc.dram_tensor(
                    "dbg_hs", [P, 4 * NCH], f32, kind="ExternalOutput"
                )
                dbg_g = nc.dram_tensor(
                    "dbg_g", [1, 32], f32, kind="ExternalOutput"
                )
                dbg_bc = nc.dram_tensor(
                    "dbg_bc", [1, BH], f32, kind="ExternalOutput"
                )
                hs_f = sb.tile([P, 2 * NCH], f32, tag="hs_f")
                hss_f = sb.tile([P, 2 * NCH], f32, tag="hss_f")
                for k in range(2):
                    nc.vector.tensor_copy(
                        hs_f[:, k * NCH : (k + 1) * NCH], hs[k][:]
                    )
                    nc.vector.tensor_copy(
                        hss_f[:, k * NCH : (k + 1) * NCH], hss[k][:]
                    )
                nc.sync.dma_start(dbg_hs.ap()[:, 0 : 2 * NCH], hs_f[:])
                nc.sync.dma_start(dbg_hs.ap()[:, 2 * NCH :], hss_f[:])
                gbuf = sb.tile([1, 32], f32, tag="gbuf")
                nc.gpsimd.memset(gbuf[:], 0.0)
                nc.vector.tensor_copy(gbuf[:, 0:E], logits[:])
                nc.vector.tensor_copy(gbuf[:, 16:24], vals8[:])
                nc.vector.tensor_copy(gbuf[:, 24:26], idx8[0:1, 0:2])
                nc.vector.tensor_copy(gbuf[:, 26:27], tkg[0][:])
                nc.vector.tensor_copy(gbuf[:, 27:28], tkg[1][:])
                nc.sync.dma_start(dbg_g.ap(), gbuf[:])
                nc.sync.dma_start(dbg_bc.ap(), bc[0][:])

    nc.compile()
    _BUILT = (nc, in_names)
    return _BUILT


def make_in_maps(x, Wg, bg, W1, b1, W2, b2):
    """Host-side sharding: per-core input dicts (weights cast to bf16)."""
    import ml_dtypes

    bf16 = ml_dtypes.bfloat16

    x = np.asarray(x, np.float32).reshape(D)
    Wg = np.asarray(Wg, np.float32)
    bg = np.asarray(bg, np.float32).reshape(E)
    W1 = np.asarray(W1, np.float32)
    b1 = np.asarray(b1, np.float32)
    W2 = np.asarray(W2, np.float32)
    b2 = np.asarray(b2, np.float32)

    # packed gating tile: [P, XW] = x chunks | Wg.T chunks | bg (row 0)
    xwg = np.zeros((P, XW), np.float32)
    xwg[:, :DC] = x.reshape(DC, P).T
    xwg[:, DC : DC + DC * E] = (
        Wg.T.reshape(DC, P, E).transpose(1, 0, 2).reshape(P, DC * E)
    )
    xwg[0, DC + DC * E :] = bg
    xwg = np.ascontiguousarray(xwg.astype(bf16))
    iota16 = np.arange(E, dtype=np.float32).reshape(1, E)

    in_maps = []
    for c in range(NCORES):
        rs = slice(c * RS, (c + 1) * RS)
        # w1b[e, p, dc*RS + r] = W1[e, c*RS + r, dc*128 + p]
        w1b = (
            W1[:, rs, :]
            .transpose(0, 2, 1)
            .reshape(E, DC, P, RS)
            .transpose(0, 2, 1, 3)
            .reshape(E, P, DC * RS)
        )
        # reference layer 2 contracts W2's LAST axis: eo = W2[e] @ h.
        # w2b[e, p, ic*H + o] = W2[e, o, c*RS + ic*128 + p]
        w2b = (
            W2[:, :, rs]
            .transpose(0, 2, 1)
            .reshape(E, NCH, P, H)
            .transpose(0, 2, 1, 3)
            .reshape(E, P, NCH * H)
        )
        bcat = np.concatenate([b1[:, rs], b2 / NCORES], axis=1)
        in_maps.append(
            {
                "xwg": xwg,
                "iota16": iota16,
                "w1b": np.ascontiguousarray(w1b.astype(bf16)),
                "w2b": np.ascontiguousarray(w2b.astype(bf16)),
                "bcat": np.ascontiguousarray(bcat),
            }
        )
    return in_maps


def combine_outs(outs):
    """Sum per-core [P, OC] partials and restore the flat [H] layout."""
    acc = np.zeros((P, OC), np.float64)
    for o in outs:
        acc += np.asarray(o, np.float32).reshape(P, OC)
    return np.ascontiguousarray(acc.T.reshape(H).astype(np.float32))


def kernel(x, Wg, bg, W1, b1, W2, b2, train=0, **_unused):
    from concourse import bass_utils

    nc, _ = _build()
    in_maps = make_in_maps(x, Wg, bg, W1, b1, W2, b2)
    res = bass_utils.run_bass_kernel_spmd(
        nc, in_maps, core_ids=list(range(NCORES))
    )
    return combine_outs([res.results[c]["out"] for c in range(NCORES)])
